# revision 16
# baseline (speedup 1.0000x reference)
import numpy as np
import jax
import jax.numpy as jnp
from jax.sharding import Mesh, NamedSharding, PartitionSpec as P

try:
    from jax.experimental.shard_map import shard_map
except ImportError:
    from jax import shard_map

# nn_GTN_58205396795517: 2-layer TransformerConv GNN.
#
# Layout: dst-sharded. Core c owns nodes [c*NPAD, (c+1)*NPAD); edges are
# bucketed to the core owning their dst, so segment-softmax and scatter-add
# are core-local. Between layers one psum replicates h [NTOT, D].
# Logits use q.k = x[dst] @ (Wq_h Wk_h^T) @ x[src]^T so only 64-wide x rows
# are gathered per edge (never 256-wide q/k rows); softmax normalization is
# deferred: out = seg(ex*v)/seg(ex).
#
# Dispatch count dominates wall time on this backend (~20-40ms/launch), so
# each layer is exactly two dispatches: one combined [src;dst] row gather
# (gathers fused with arith produce NEFFs that wedge the device, so the
# gather stays isolated) and one fused everything-else stage.
N = 100000
E = 800000
D = 64
H = 4
C = 64
M = 8
NPAD = 12544            # per-core node count (98*128)
NTOT = NPAD * M         # 100352

_INV_SQRT_C = np.float32(1.0 / np.sqrt(C))

_state = None


def _build():
    global _state
    if _state is not None:
        return _state
    mesh = Mesh(np.array(jax.devices()[:M]), ('x',))
    rep = NamedSharding(mesh, P())
    esh = NamedSharding(mesh, P('x'))

    def smap(fn, in_specs, out_specs):
        return jax.jit(shard_map(fn, mesh=mesh, in_specs=in_specs,
                                 out_specs=out_specs))

    J = {}
    # one combined row gather from the replicated node table
    J['gat'] = smap(lambda t, i: t[i], (P(), P('x')), P('x'))

    # everything else for one layer, fused: projections, logits+exp,
    # messages, local segment sums, normalize+skip (+relu, +psum-replicate)
    def _rest(g, x_own_, locs, basis, CW, U, W2, c0, bv, Ws, bs,
              do_relu, do_ag):
        ep = g.shape[0] // 2
        xs, xd = g[:ep], g[ep:]
        yv = xs @ CW
        ex = jnp.exp(((yv[:, :H * D].reshape(-1, H, D)
                       * xd[:, None, :]).sum(-1)
                      + xd @ U + xs @ W2 + c0) * _INV_SQRT_C)
        m = (yv[:, H * D:] + bv) * jnp.repeat(ex, C, axis=1)
        agg = jax.ops.segment_sum(m, locs, num_segments=NPAD + 1)[:NPAD]
        den = jax.ops.segment_sum(ex, locs, num_segments=NPAD + 1)[:NPAD]
        out = (agg.reshape(NPAD, H, C)
               / (den + 1e-16)[:, :, None]).mean(axis=1) + x_own_ @ Ws + bs
        if do_relu:
            out = jax.nn.relu(out)
        if do_ag:
            out_rep = jax.lax.psum(
                (basis.reshape(M, 1, 1) * out[None, :, :]).reshape(NTOT, D),
                'x')
            return out, out_rep
        return out

    rest_specs = (P('x'), P('x'), P('x'), P('x')) + (P(),) * 7
    J['rest1'] = smap(lambda *a: _rest(*a, True, True),
                      rest_specs, (P('x'), P()))
    J['rest2'] = smap(lambda *a: _rest(*a, False, False),
                      rest_specs, P('x'))

    _state = (mesh, rep, esh, J)
    return _state


def _prep_weights(Wq, bq, Wk, bk, Wv, bv):
    """Host-side exact refactor of the per-head q.k dot into x-space."""
    Wq = np.asarray(Wq, np.float32); Wk = np.asarray(Wk, np.float32)
    Wv = np.asarray(Wv, np.float32)
    bq = np.asarray(bq, np.float32); bk = np.asarray(bk, np.float32)
    Cs, Us, W2s, c0s = [], [], [], []
    for h in range(H):
        Wq_h = Wq[:, h * C:(h + 1) * C]
        Wk_h = Wk[:, h * C:(h + 1) * C]
        bq_h = bq[h * C:(h + 1) * C]
        bk_h = bk[h * C:(h + 1) * C]
        Cs.append(Wk_h @ Wq_h.T)        # y_h = xs @ C_h ; alpha_h = y_h . xd
        Us.append(Wq_h @ bk_h)          # xd-linear bias term
        W2s.append(Wk_h @ bq_h)         # xs-linear bias term
        c0s.append(bq_h @ bk_h)
    CW = np.concatenate(Cs + [Wv], axis=1)              # [D, H*D + H*C]
    U = np.stack(Us, axis=1)                            # [D, H]
    W2 = np.stack(W2s, axis=1)                          # [D, H]
    c0 = np.array(c0s, np.float32)[None, :]             # [1, H]
    return CW, U, W2, c0, np.asarray(bv, np.float32)


def _prep_edges(edge_index):
    """Sort edges by dst, bucket by owning core, pad to a common length.
    Returns the combined [src; dst_global] gather index array and the
    segment index array (pad edges dump into segment NPAD)."""
    ei = np.asarray(edge_index)
    src, dst = ei[0].astype(np.int64), ei[1].astype(np.int64)
    order = np.argsort(dst, kind='stable')
    src, dst = src[order], dst[order]
    core = dst // NPAD
    counts = np.bincount(core, minlength=M)
    epad = int(-(-counts.max() // 128) * 128)
    gidx = np.zeros((M, 2 * epad), np.int32)     # [src rows | dst rows]
    locs = np.full((M, epad), NPAD, np.int32)    # segment idx: pad -> dump
    start = 0
    for c in range(M):
        n = int(counts[c])
        gidx[c, :n] = src[start:start + n]
        gidx[c, epad:epad + n] = dst[start:start + n]
        gidx[c, epad + n:] = c * NPAD            # in-bounds pad (own row 0)
        locs[c, :n] = dst[start:start + n] - c * NPAD
        start += n
    return gidx.reshape(-1), locs.reshape(-1), epad


def kernel(x, edge_index, Wq1, bq1, Wk1, bk1, Wv1, bv1, Ws1, bs1,
           Wq2, bq2, Wk2, bk2, Wv2, bv2, Ws2, bs2):
    mesh, rep, esh, J = _build()

    gidx_a, locs_a, _ = _prep_edges(edge_index)
    gidx = jax.device_put(jnp.asarray(gidx_a), esh)
    locs = jax.device_put(jnp.asarray(locs_a), esh)

    xp = np.zeros((NTOT, D), np.float32)
    xp[:N] = np.asarray(x, np.float32)
    x_own = jax.device_put(jnp.asarray(xp), esh)

    pr = lambda a: jax.device_put(jnp.asarray(np.asarray(a, np.float32)), rep)
    w1 = tuple(map(pr, _prep_weights(Wq1, bq1, Wk1, bk1, Wv1, bv1))) \
        + (pr(Ws1), pr(bs1))
    w2 = tuple(map(pr, _prep_weights(Wq2, bq2, Wk2, bk2, Wv2, bv2))) \
        + (pr(Ws2), pr(bs2))
    basis = jax.device_put(jnp.asarray(np.eye(M, dtype=np.float32)), esh)

    # layer-1 gather has static input: do it on the host (input prep),
    # saving one of the two expensive device gather dispatches
    g1_host = xp[gidx_a]                       # [M*2EP, D]
    g1 = jax.device_put(jnp.asarray(g1_host), esh)
    h_own, h_rep = J['rest1'](g1, x_own, locs, basis, *w1)
    g2 = J['gat'](h_rep, gidx)
    out_own = J['rest2'](g2, h_own, locs, basis, *w2)
    out = np.asarray(jax.device_get(out_own))
    return out[:N].astype(np.float32)


# revision 17
# speedup vs baseline: 1.0341x; 1.0341x over previous
import numpy as np
import jax
import jax.numpy as jnp
from jax.sharding import Mesh, NamedSharding, PartitionSpec as P

try:
    from jax.experimental.shard_map import shard_map
except ImportError:
    from jax import shard_map

# nn_GTN_58205396795517: 2-layer TransformerConv GNN.
#
# Layout: dst-sharded. Core c owns nodes [c*NPAD, (c+1)*NPAD); edges are
# bucketed to the core owning their dst, so segment-softmax and scatter-add
# are core-local. Between layers one psum replicates h [NTOT, D].
# Logits use q.k = x[dst] @ (Wq_h Wk_h^T) @ x[src]^T so only 64-wide x rows
# are gathered per edge (never 256-wide q/k rows); softmax normalization is
# deferred: out = seg(ex*v)/seg(ex).
#
# Dispatch count dominates wall time on this backend (~20-40ms/launch), so
# each layer is exactly two dispatches: one combined [src;dst] row gather
# (gathers fused with arith produce NEFFs that wedge the device, so the
# gather stays isolated) and one fused everything-else stage.
N = 100000
E = 800000
D = 64
H = 4
C = 64
M = 8
NPAD = 12544            # per-core node count (98*128)
NTOT = NPAD * M         # 100352

_INV_SQRT_C = np.float32(1.0 / np.sqrt(C))

_state = None


def _build():
    global _state
    if _state is not None:
        return _state
    mesh = Mesh(np.array(jax.devices()[:M]), ('x',))
    rep = NamedSharding(mesh, P())
    esh = NamedSharding(mesh, P('x'))

    def smap(fn, in_specs, out_specs):
        return jax.jit(shard_map(fn, mesh=mesh, in_specs=in_specs,
                                 out_specs=out_specs))

    J = {}
    # one combined row gather from the replicated node table
    J['gat'] = smap(lambda t, i: t[i], (P(), P('x')), P('x'))

    # everything else for one layer, fused: projections, logits+exp,
    # messages, local segment sums, normalize+skip (+relu, +psum-replicate)
    def _rest(g, x_own_, locs, basis, CW, U, W2, c0, bv, Ws, bs,
              do_relu, do_ag):
        ep = g.shape[0] // 2
        xs, xd = g[:ep], g[ep:]
        yv = xs @ CW
        ex = jnp.exp(((yv[:, :H * D].reshape(-1, H, D)
                       * xd[:, None, :]).sum(-1)
                      + xd @ U + xs @ W2 + c0) * _INV_SQRT_C)
        m = (yv[:, H * D:] + bv) * jnp.repeat(ex, C, axis=1)
        agg = jax.ops.segment_sum(m, locs, num_segments=NPAD + 1)[:NPAD]
        den = jax.ops.segment_sum(ex, locs, num_segments=NPAD + 1)[:NPAD]
        out = (agg.reshape(NPAD, H, C)
               / (den + 1e-16)[:, :, None]).mean(axis=1) + x_own_ @ Ws + bs
        if do_relu:
            out = jax.nn.relu(out)
        if do_ag:
            out_rep = jax.lax.psum(
                (basis.reshape(M, 1, 1) * out[None, :, :]).reshape(NTOT, D),
                'x')
            return out, out_rep
        return out

    # rest1 + the layer-2 gather fused at the end (h_rep is live post-psum)
    def _rest1g(g, x_own_, locs, basis, gidx_, CW, U, W2, c0, bv, Ws, bs):
        out, out_rep = _rest(g, x_own_, locs, basis, CW, U, W2, c0, bv,
                             Ws, bs, True, True)
        return out, out_rep[gidx_]

    rest_specs = (P('x'), P('x'), P('x'), P('x')) + (P(),) * 7
    J['rest1'] = smap(lambda *a: _rest(*a, True, True),
                      rest_specs, (P('x'), P()))
    J['rest2'] = smap(lambda *a: _rest(*a, False, False),
                      rest_specs, P('x'))
    J['rest1g'] = smap(_rest1g,
                       (P('x'), P('x'), P('x'), P('x'), P('x')) + (P(),) * 7,
                       (P('x'), P('x')))

    _state = (mesh, rep, esh, J)
    return _state


def _prep_weights(Wq, bq, Wk, bk, Wv, bv):
    """Host-side exact refactor of the per-head q.k dot into x-space."""
    Wq = np.asarray(Wq, np.float32); Wk = np.asarray(Wk, np.float32)
    Wv = np.asarray(Wv, np.float32)
    bq = np.asarray(bq, np.float32); bk = np.asarray(bk, np.float32)
    Cs, Us, W2s, c0s = [], [], [], []
    for h in range(H):
        Wq_h = Wq[:, h * C:(h + 1) * C]
        Wk_h = Wk[:, h * C:(h + 1) * C]
        bq_h = bq[h * C:(h + 1) * C]
        bk_h = bk[h * C:(h + 1) * C]
        Cs.append(Wk_h @ Wq_h.T)        # y_h = xs @ C_h ; alpha_h = y_h . xd
        Us.append(Wq_h @ bk_h)          # xd-linear bias term
        W2s.append(Wk_h @ bq_h)         # xs-linear bias term
        c0s.append(bq_h @ bk_h)
    CW = np.concatenate(Cs + [Wv], axis=1)              # [D, H*D + H*C]
    U = np.stack(Us, axis=1)                            # [D, H]
    W2 = np.stack(W2s, axis=1)                          # [D, H]
    c0 = np.array(c0s, np.float32)[None, :]             # [1, H]
    return CW, U, W2, c0, np.asarray(bv, np.float32)


def _prep_edges(edge_index):
    """Sort edges by dst, bucket by owning core, pad to a common length.
    Returns the combined [src; dst_global] gather index array and the
    segment index array (pad edges dump into segment NPAD)."""
    ei = np.asarray(edge_index)
    src, dst = ei[0].astype(np.int64), ei[1].astype(np.int64)
    order = np.argsort(dst, kind='stable')
    src, dst = src[order], dst[order]
    core = dst // NPAD
    counts = np.bincount(core, minlength=M)
    epad = int(-(-counts.max() // 128) * 128)
    gidx = np.zeros((M, 2 * epad), np.int32)     # [src rows | dst rows]
    locs = np.full((M, epad), NPAD, np.int32)    # segment idx: pad -> dump
    start = 0
    for c in range(M):
        n = int(counts[c])
        gidx[c, :n] = src[start:start + n]
        gidx[c, epad:epad + n] = dst[start:start + n]
        gidx[c, epad + n:] = c * NPAD            # in-bounds pad (own row 0)
        locs[c, :n] = dst[start:start + n] - c * NPAD
        start += n
    return gidx.reshape(-1), locs.reshape(-1), epad


def kernel(x, edge_index, Wq1, bq1, Wk1, bk1, Wv1, bv1, Ws1, bs1,
           Wq2, bq2, Wk2, bk2, Wv2, bv2, Ws2, bs2):
    mesh, rep, esh, J = _build()

    gidx_a, locs_a, _ = _prep_edges(edge_index)
    gidx = jax.device_put(jnp.asarray(gidx_a), esh)
    locs = jax.device_put(jnp.asarray(locs_a), esh)

    xp = np.zeros((NTOT, D), np.float32)
    xp[:N] = np.asarray(x, np.float32)
    x_own = jax.device_put(jnp.asarray(xp), esh)

    pr = lambda a: jax.device_put(jnp.asarray(np.asarray(a, np.float32)), rep)
    w1 = tuple(map(pr, _prep_weights(Wq1, bq1, Wk1, bk1, Wv1, bv1))) \
        + (pr(Ws1), pr(bs1))
    w2 = tuple(map(pr, _prep_weights(Wq2, bq2, Wk2, bk2, Wv2, bv2))) \
        + (pr(Ws2), pr(bs2))
    basis = jax.device_put(jnp.asarray(np.eye(M, dtype=np.float32)), esh)

    # layer-1 gather has static input: do it on the host (input prep),
    # saving one of the two expensive device gather dispatches
    g1_host = xp[gidx_a]                       # [M*2EP, D]
    g1 = jax.device_put(jnp.asarray(g1_host), esh)
    h_own, g2 = J['rest1g'](g1, x_own, locs, basis, gidx, *w1)
    out_own = J['rest2'](g2, h_own, locs, basis, *w2)
    out = np.asarray(jax.device_get(out_own))
    return out[:N].astype(np.float32)


# revision 18
# speedup vs baseline: 1.2163x; 1.1762x over previous
import numpy as np
import jax
import jax.numpy as jnp
from jax.sharding import Mesh, NamedSharding, PartitionSpec as P

try:
    from jax.experimental.shard_map import shard_map
except ImportError:
    from jax import shard_map

# nn_GTN_58205396795517: 2-layer TransformerConv GNN.
#
# Layout: dst-sharded. Core c owns nodes [c*NPAD, (c+1)*NPAD); edges are
# bucketed to the core owning their dst, so segment-softmax and scatter-add
# are core-local. Between layers one psum replicates h [NTOT, D].
# Logits use q.k = x[dst] @ (Wq_h Wk_h^T) @ x[src]^T so only 64-wide x rows
# are gathered per edge (never 256-wide q/k rows); softmax normalization is
# deferred: out = seg(ex*v)/seg(ex).
#
# Dispatch count dominates wall time on this backend (~20-40ms/launch), so
# each layer is exactly two dispatches: one combined [src;dst] row gather
# (gathers fused with arith produce NEFFs that wedge the device, so the
# gather stays isolated) and one fused everything-else stage.
N = 100000
E = 800000
D = 64
H = 4
C = 64
M = 8
NPAD = 12544            # per-core node count (98*128)
NTOT = NPAD * M         # 100352

_INV_SQRT_C = np.float32(1.0 / np.sqrt(C))

_state = None


def _build():
    global _state
    if _state is not None:
        return _state
    mesh = Mesh(np.array(jax.devices()[:M]), ('x',))
    rep = NamedSharding(mesh, P())
    esh = NamedSharding(mesh, P('x'))

    def smap(fn, in_specs, out_specs):
        return jax.jit(shard_map(fn, mesh=mesh, in_specs=in_specs,
                                 out_specs=out_specs))

    J = {}
    # one combined row gather from the replicated node table
    J['gat'] = smap(lambda t, i: t[i], (P(), P('x')), P('x'))

    # everything else for one layer, fused: projections, logits+exp,
    # messages, local segment sums, normalize+skip (+relu, +psum-replicate)
    def _rest(g, x_own_, locs, basis, CW, U, W2, c0, bv, Ws, bs,
              do_relu, do_ag):
        ep = g.shape[0] // 2
        xs, xd = g[:ep], g[ep:]
        yv = xs @ CW
        ex = jnp.exp(((yv[:, :H * D].reshape(-1, H, D)
                       * xd[:, None, :]).sum(-1)
                      + xd @ U + xs @ W2 + c0) * _INV_SQRT_C)
        mv = (yv[:, H * D:].reshape(-1, H, C)
              + bv.reshape(H, C)[None]) * ex[:, :, None]
        cat = jnp.concatenate([mv.reshape(-1, H * C), ex], axis=1)
        s = jax.ops.segment_sum(cat, locs, num_segments=NPAD + 1)[:NPAD]
        out = (s[:, :H * C].reshape(NPAD, H, C)
               / (s[:, H * C:] + 1e-16)[:, :, None]).mean(axis=1) \
            + x_own_ @ Ws + bs
        if do_relu:
            out = jax.nn.relu(out)
        if do_ag:
            out_rep = jax.lax.psum(
                (basis.reshape(M, 1, 1) * out[None, :, :]).reshape(NTOT, D),
                'x')
            return out, out_rep
        return out

    # rest1 + the layer-2 gather fused at the end (h_rep is live post-psum)
    def _rest1g(g, x_own_, locs, basis, gidx_, CW, U, W2, c0, bv, Ws, bs):
        out, out_rep = _rest(g, x_own_, locs, basis, CW, U, W2, c0, bv,
                             Ws, bs, True, True)
        return out, out_rep[gidx_]

    rest_specs = (P('x'), P('x'), P('x'), P('x')) + (P(),) * 7
    J['rest1'] = smap(lambda *a: _rest(*a, True, True),
                      rest_specs, (P('x'), P()))
    J['rest2'] = smap(lambda *a: _rest(*a, False, False),
                      rest_specs, P('x'))
    J['rest1g'] = smap(_rest1g,
                       (P('x'), P('x'), P('x'), P('x'), P('x')) + (P(),) * 7,
                       (P('x'), P('x')))

    _state = (mesh, rep, esh, J)
    return _state


def _prep_weights(Wq, bq, Wk, bk, Wv, bv):
    """Host-side exact refactor of the per-head q.k dot into x-space."""
    Wq = np.asarray(Wq, np.float32); Wk = np.asarray(Wk, np.float32)
    Wv = np.asarray(Wv, np.float32)
    bq = np.asarray(bq, np.float32); bk = np.asarray(bk, np.float32)
    Cs, Us, W2s, c0s = [], [], [], []
    for h in range(H):
        Wq_h = Wq[:, h * C:(h + 1) * C]
        Wk_h = Wk[:, h * C:(h + 1) * C]
        bq_h = bq[h * C:(h + 1) * C]
        bk_h = bk[h * C:(h + 1) * C]
        Cs.append(Wk_h @ Wq_h.T)        # y_h = xs @ C_h ; alpha_h = y_h . xd
        Us.append(Wq_h @ bk_h)          # xd-linear bias term
        W2s.append(Wk_h @ bq_h)         # xs-linear bias term
        c0s.append(bq_h @ bk_h)
    CW = np.concatenate(Cs + [Wv], axis=1)              # [D, H*D + H*C]
    U = np.stack(Us, axis=1)                            # [D, H]
    W2 = np.stack(W2s, axis=1)                          # [D, H]
    c0 = np.array(c0s, np.float32)[None, :]             # [1, H]
    return CW, U, W2, c0, np.asarray(bv, np.float32)


def _prep_edges(edge_index):
    """Sort edges by dst, bucket by owning core, pad to a common length.
    Returns the combined [src; dst_global] gather index array and the
    segment index array (pad edges dump into segment NPAD)."""
    ei = np.asarray(edge_index)
    src, dst = ei[0].astype(np.int64), ei[1].astype(np.int64)
    order = np.argsort(dst, kind='stable')
    src, dst = src[order], dst[order]
    core = dst // NPAD
    counts = np.bincount(core, minlength=M)
    epad = int(-(-counts.max() // 128) * 128)
    gidx = np.zeros((M, 2 * epad), np.int32)     # [src rows | dst rows]
    locs = np.full((M, epad), NPAD, np.int32)    # segment idx: pad -> dump
    start = 0
    for c in range(M):
        n = int(counts[c])
        gidx[c, :n] = src[start:start + n]
        gidx[c, epad:epad + n] = dst[start:start + n]
        gidx[c, epad + n:] = c * NPAD            # in-bounds pad (own row 0)
        locs[c, :n] = dst[start:start + n] - c * NPAD
        start += n
    return gidx.reshape(-1), locs.reshape(-1), epad


def kernel(x, edge_index, Wq1, bq1, Wk1, bk1, Wv1, bv1, Ws1, bs1,
           Wq2, bq2, Wk2, bk2, Wv2, bv2, Ws2, bs2):
    mesh, rep, esh, J = _build()

    gidx_a, locs_a, _ = _prep_edges(edge_index)
    gidx = jax.device_put(jnp.asarray(gidx_a), esh)
    locs = jax.device_put(jnp.asarray(locs_a), esh)

    xp = np.zeros((NTOT, D), np.float32)
    xp[:N] = np.asarray(x, np.float32)
    x_own = jax.device_put(jnp.asarray(xp), esh)

    pr = lambda a: jax.device_put(jnp.asarray(np.asarray(a, np.float32)), rep)
    w1 = tuple(map(pr, _prep_weights(Wq1, bq1, Wk1, bk1, Wv1, bv1))) \
        + (pr(Ws1), pr(bs1))
    w2 = tuple(map(pr, _prep_weights(Wq2, bq2, Wk2, bk2, Wv2, bv2))) \
        + (pr(Ws2), pr(bs2))
    basis = jax.device_put(jnp.asarray(np.eye(M, dtype=np.float32)), esh)

    # layer-1 gather has static input: do it on the host (input prep),
    # saving one of the two expensive device gather dispatches
    g1_host = xp[gidx_a]                       # [M*2EP, D]
    g1 = jax.device_put(jnp.asarray(g1_host), esh)
    h_own, g2 = J['rest1g'](g1, x_own, locs, basis, gidx, *w1)
    out_own = J['rest2'](g2, h_own, locs, basis, *w2)
    out = np.asarray(jax.device_get(out_own))
    return out[:N].astype(np.float32)
